# revision 6
# baseline (speedup 1.0000x reference)
"""Trainium2 Bass kernel for CausalEVAttention (sparse_attention).

Sharding: 8 cores = 4 batches x 2 head-groups (8 heads each).
Each core computes QKV projections (fp16 matmuls), windowed local causal
attention + EVA random-feature chunk branch, and a partial output
projection over its head group.  Host sums the two head-group partials
per batch and adds the output bias.

All heavy matmuls run in fp16 (inputs pre-cast on host); accumulation is
fp32 in PSUM.  Softmax runs without max-subtraction (logit magnitudes are
small); masked lanes use -1e9 biases.  The beta (within-chunk) softmax is
folded into the joint softmax: rfa value blocks stay unnormalized and the
pseudo-key logits get a -ln(sum) per-partition bias.
"""

import numpy as np

import concourse.bass as bass
import concourse.mybir as mybir
import concourse.tile as tile
from concourse import bacc
from concourse.bass_utils import run_bass_kernel_spmd

dt = mybir.dt
AF = mybir.ActivationFunctionType
ALU = mybir.AluOpType

N, B, E, H = 4096, 4, 1024, 16
D = 64                # head dim
HPC = 8               # heads per core
G = 32                # windows (128 queries each)
C = 32                # rf chunks (128 keys each)
W = 128               # window size
SCALE = D ** -0.5     # 0.125
NEG = -1e9

_CACHED = {}


def _build_nc():
    nc = bacc.Bacc("TRN2", target_bir_lowering=False, debug=False, num_devices=8)

    f16, f32 = dt.float16, dt.float32
    inp = lambda name, shape, d: nc.dram_tensor(name, shape, d, kind="ExternalInput").ap()

    xt = inp("xt", [E, N], f16)               # query[:, b, :].T
    wt = inp("wt", [E, 3 * 512], f16)         # [WqT | WkT | WvT] head-group slice
    bqk = inp("bqk", [128, 8], f32)           # packed (bq*0.125 | bk) per m-tile
    bvrow = inp("bvrow", [1, 512], f16)
    wot = inp("wot", [512, E], f16)           # Wo[:, hs].T
    muqw = inp("muqw", [D, D], f16)           # mu_q_w.T / 128
    mukw = inp("mukw", [D, D], f16)
    mubq_bc = inp("mubq_bc", [128, D], f32)   # mu_q_b broadcast
    mubk_bc = inp("mubk_bc", [128, D], f32)
    lnconst = inp("lnconst", [128, 4 * D], f32)  # [gq | beq | gk | bek] broadcasts
    mask01 = inp("mask01", [128, 128], f16)   # causal keep-mask (S^T diag block)
    maskbias = inp("maskbias", [32, 32], f32)  # chunk-visibility bias [c, g]
    ident16 = inp("ident16", [128, 128], f16)
    ident32 = inp("ident32", [128, 128], f32)
    neghalf = inp("neghalf", [128, 1], f16)   # -scale/2
    ones16 = inp("ones16", [1, 128], f16)
    epscol = inp("epscol", [128, 1], f32)
    onesv = inp("onesv", [128, C * HPC], f16)  # ones for v_aug 65th column

    outT = nc.dram_tensor("outT", [E, N], f16, kind="ExternalOutput").ap()
    aot = nc.dram_tensor("aot", [512, N], f16).ap()   # attn output^T staging (DRAM)

    from contextlib import ExitStack
    with tile.TileContext(nc) as tc, ExitStack() as stk:
        cpool = stk.enter_context(tc.tile_pool(name="consts", bufs=1))
        bigp = stk.enter_context(tc.tile_pool(name="bigs", bufs=1))
        wkp = stk.enter_context(tc.tile_pool(name="work", bufs=2))
        psum = stk.enter_context(tc.tile_pool(name="ps", bufs=1, space="PSUM"))

        # ---------------- constants / weights ----------------
        wt_sb = cpool.tile([128, 8, 3 * 512], f16)
        nc.sync.dma_start(wt_sb[:], wt.rearrange("(k p) m -> p k m", p=128))
        wot_sb = cpool.tile([128, 4, E], f16)
        nc.sync.dma_start(wot_sb[:], wot.rearrange("(k p) m -> p k m", p=128))
        bqk_sb = cpool.tile([128, 8], f32)
        nc.sync.dma_start(bqk_sb[:], bqk)
        bvrow_sb = cpool.tile([1, 512], f16)
        nc.sync.dma_start(bvrow_sb[:], bvrow)
        muqw_sb = cpool.tile([128, D], f16)   # duplicated across halves
        nc.sync.dma_start(muqw_sb[0:64, :], muqw)
        nc.sync.dma_start(muqw_sb[64:128, :], muqw)
        mukw_sb = cpool.tile([128, D], f16)
        nc.sync.dma_start(mukw_sb[0:64, :], mukw)
        nc.sync.dma_start(mukw_sb[64:128, :], mukw)
        mubq_sb = cpool.tile([128, D], f32)
        nc.sync.dma_start(mubq_sb[:], mubq_bc)
        mubk_sb = cpool.tile([128, D], f32)
        nc.sync.dma_start(mubk_sb[:], mubk_bc)
        lnc_sb = cpool.tile([128, 4 * D], f32)
        nc.sync.dma_start(lnc_sb[:], lnconst)
        mask_sb = cpool.tile([128, 128], f16)
        nc.sync.dma_start(mask_sb[:], mask01)
        mbias_sb = cpool.tile([32, 32], f32)
        nc.sync.dma_start(mbias_sb[:], maskbias)
        id16_sb = cpool.tile([128, 128], f16)
        nc.sync.dma_start(id16_sb[:], ident16)
        id32_sb = cpool.tile([128, 128], f32)
        nc.sync.dma_start(id32_sb[:], ident32)
        ngh_sb = cpool.tile([128, 1], f16)
        nc.sync.dma_start(ngh_sb[:], neghalf)
        ones_sb = cpool.tile([1, 128], f16)
        nc.sync.dma_start(ones_sb[:], ones16)
        eps_sb = cpool.tile([128, 1], f32)
        nc.sync.dma_start(eps_sb[:], epscol)
        # ones row living at partition 64 (lhsT mate for the row-64 reciprocal)
        ones_r64 = cpool.tile([65, 128], f16)
        nc.sync.dma_start(ones_r64[64:65, :], ones16)

        # ---------------- big persistent tensors ----------------
        qT = [bigp.tile([128, N], f16, tag=f"qT{t}", name=f"qT{t}") for t in range(4)]
        kT = [bigp.tile([128, N], f16, tag=f"kT{t}", name=f"kT{t}") for t in range(4)]
        kT2 = [bigp.tile([128, N], f16, tag=f"kT2{t}", name=f"kT2{t}") for t in range(4)]
        v_aug = bigp.tile([128, C, HPC, D + 1], f16)
        nc.sync.dma_start(v_aug[:, :, :, D], onesv)

        # ---------------- phase 1: QKV projections ----------------
        for ns in range(8):
            nsl = slice(ns * 512, (ns + 1) * 512)
            xs = wkp.tile([128, 8, 512], f16, tag="xs")
            nc.sync.dma_start(xs[:], xt.rearrange("(k p) n -> p k n", p=128)[:, :, nsl])
            for m in range(8):
                ps = psum.tile([128, 512], f32, tag="big", bufs=2)
                for k in range(8):
                    nc.tensor.matmul(ps[:], wt_sb[:, k, m * 128:(m + 1) * 128],
                                     xs[:, k, :], start=(k == 0), stop=(k == 7))
                if m < 4:
                    nc.scalar.activation(qT[m][:, nsl], ps[:], AF.Identity,
                                         bias=bqk_sb[:, m:m + 1], scale=SCALE)
                else:
                    nc.scalar.activation(kT[m - 4][:, nsl], ps[:], AF.Identity,
                                         bias=bqk_sb[:, m:m + 1], scale=1.0)
            for nb in range(4):
                g = ns * 4 + nb
                ps = psum.tile([128, 512], f32, tag="big", bufs=2)
                for k in range(8):
                    nc.tensor.matmul(ps[:], xs[:, k, nb * 128:(nb + 1) * 128],
                                     wt_sb[:, k, 1024:1536], start=(k == 0), stop=False)
                nc.tensor.matmul(ps[:], ones_sb[0:1, :], bvrow_sb[0:1, :],
                                 start=False, stop=True)
                nc.scalar.copy(v_aug[:, g, :, 0:D], ps[:].rearrange("p (h d) -> p h d", d=D))

        # ---------------- phase 2: RFA statistics ----------------
        meansQ = wkp.tile([128, 4, C], f32, tag="meansQ", bufs=1)
        meansK = wkp.tile([128, 4, C], f32, tag="meansK", bufs=1)
        for t in range(4):
            nc.vector.tensor_reduce(out=meansQ[:, t, :],
                                    in_=qT[t][:].rearrange("p (c w) -> p c w", w=W),
                                    op=ALU.add, axis=mybir.AxisListType.X)
            nc.vector.tensor_reduce(out=meansK[:, t, :],
                                    in_=kT[t][:].rearrange("p (c w) -> p c w", w=W),
                                    op=ALU.add, axis=mybir.AxisListType.X)
        meansQ16 = wkp.tile([128, 4, C], f16, tag="mQ16", bufs=1)
        meansK16 = wkp.tile([128, 4, C], f16, tag="mK16", bufs=1)
        nc.scalar.copy(meansQ16[:], meansQ[:])
        nc.scalar.copy(meansK16[:], meansK[:])

        # per-head linear + layernorm (both sides), then mu = qbar + kbar
        mu_pack = wkp.tile([128, 128], f32, tag="mu_pack", bufs=1)
        rfk_pack = wkp.tile([128, 128], f32, tag="rfk_pack", bufs=1)
        for h in range(HPC):
            t, b64 = h // 2, 64 * (h % 2)
            jr, jc = h // 2, h % 2
            bars = []
            for side in range(2):  # 0 = q, 1 = k
                mw = muqw_sb if side == 0 else mukw_sb
                mean16 = meansQ16 if side == 0 else meansK16
                mub = mubq_sb if side == 0 else mubk_sb
                gofs = 0 if side == 0 else 2 * D
                psl = psum.tile([32, D], f32, tag="small", bufs=4)
                nc.tensor.matmul(psl[:], mean16[b64:b64 + 64, t, :],
                                 mw[b64:b64 + 64, :], start=True, stop=True)
                x = wkp.tile([32, D], f32, tag=f"lnx{side}", bufs=2)
                nc.vector.tensor_tensor(out=x[:], in0=psl[:], in1=mub[0:32, :], op=ALU.add)
                mn = wkp.tile([32, 1], f32, tag=f"lnm{side}", bufs=2)
                nc.vector.tensor_reduce(out=mn[:], in_=x[:], op=ALU.add,
                                        axis=mybir.AxisListType.X)
                nc.vector.tensor_scalar_mul(mn[:], mn[:], 1.0 / D)
                nc.vector.tensor_scalar(out=x[:], in0=x[:], scalar1=mn[:],
                                        scalar2=None, op0=ALU.subtract)
                junk = wkp.tile([32, D], f32, tag="junk", bufs=2)
                var = wkp.tile([32, 1], f32, tag=f"lnv{side}", bufs=2)
                nc.scalar.activation(junk[:], x[:], AF.Square, scale=float(D ** -0.5),
                                     accum_out=var[:])
                nc.scalar.activation(var[:], var[:], AF.Sqrt, bias=eps_sb[0:32, :])
                nc.vector.reciprocal(var[:], var[:])
                nc.vector.tensor_scalar_mul(x[:], x[:], var[:])
                bar = wkp.tile([32, D], f32, tag=f"bar{side}", bufs=2)
                nc.vector.scalar_tensor_tensor(out=bar[:], in0=x[:], scalar=1.0,
                                               in1=lnc_sb[0:32, gofs:gofs + D],
                                               op0=ALU.mult, op1=ALU.mult)
                nc.vector.tensor_tensor(out=bar[:], in0=bar[:],
                                        in1=lnc_sb[0:32, gofs + D:gofs + 2 * D], op=ALU.add)
                bars.append(bar)
            mu_h = wkp.tile([32, D], f32, tag="mu_h", bufs=2)
            nc.vector.tensor_tensor(out=mu_h[:], in0=bars[0][:], in1=bars[1][:], op=ALU.add)
            nc.sync.dma_start(mu_pack[32 * jr:32 * jr + 32, 64 * jc:64 * jc + 64], mu_h[:])
            nc.sync.dma_start(rfk_pack[32 * jr:32 * jr + 32, 64 * jc:64 * jc + 64], bars[1][:])

        # transpose packs -> muT16 (scaled), rfkbT16
        muT16 = wkp.tile([128, 128], f16, tag="muT16", bufs=1)
        rfkbT16 = wkp.tile([128, 128], f16, tag="rfkbT16", bufs=1)
        pst = psum.tile([128, 128], f32, tag="smallb", bufs=2)
        nc.tensor.transpose(pst[:], mu_pack[:], id32_sb[:])
        nc.scalar.activation(muT16[:], pst[:], AF.Copy, scale=SCALE)
        pst2 = psum.tile([128, 128], f32, tag="smallb", bufs=2)
        nc.tensor.transpose(pst2[:], rfk_pack[:], id32_sb[:])
        nc.scalar.copy(rfkbT16[:], pst2[:])

        # kT squared (for the -|k|^2/2 term)
        for t in range(4):
            nc.vector.tensor_tensor(out=kT2[t][:], in0=kT[t][:], in1=kT[t][:], op=ALU.mult)

        # per-head chunk logits, exp, unnormalized U, bias table
        rfa_aug = wkp.tile([32, HPC, D + 1], f16, tag="rfa_aug", bufs=1)
        bias_all = wkp.tile([32, HPC, G], f32, tag="bias_all", bufs=1)
        for h in range(HPC):
            t, b64 = h // 2, 64 * (h % 2)
            ch = 32 * (h // 2)
            hsl = slice(b64, b64 + 64)
            pslp = psum.tile([128, C], f32, tag="small", bufs=4)
            for c in range(C):
                csl = slice(c * W, (c + 1) * W)
                nc.tensor.matmul(pslp[:, c:c + 1], kT[t][hsl, csl],
                                 muT16[hsl, ch + c:ch + c + 1], start=True, stop=False)
                nc.tensor.matmul(pslp[:, c:c + 1], kT2[t][hsl, csl],
                                 ngh_sb[hsl, :], start=False, stop=True)
            explp = wkp.tile([128, C], f16, tag="explp", bufs=2)
            nc.scalar.activation(explp[:], pslp[:], AF.Exp)
            psu = psum.tile([D + 1, C], f32, tag="small", bufs=4)
            for c in range(C):
                nc.tensor.matmul(psu[:, c:c + 1], v_aug[:, c, h, :],
                                 explp[:, c:c + 1], start=True, stop=True)
            u16 = wkp.tile([D + 1, C], f16, tag="u16", bufs=2)
            nc.scalar.activation(u16[:], psu[:], AF.Copy, scale=1.0 / 16)
            psut = psum.tile([C, D + 1], f16, tag="smallb", bufs=2)
            nc.tensor.transpose(psut[:], u16[:], id16_sb[0:D + 1, 0:D + 1])
            nc.scalar.copy(rfa_aug[:, h, :], psut[:])
            lns = wkp.tile([32, 1], f32, tag="lns", bufs=2)
            nc.scalar.activation(lns[:], rfa_aug[:, h, D:D + 1], AF.Ln)
            nc.vector.tensor_scalar(out=bias_all[:, h, :], in0=mbias_sb[:],
                                    scalar1=lns[:], scalar2=None, op0=ALU.subtract)

        # ---------------- phase 3: windowed attention ----------------
        for h in range(HPC):
            t, b64 = h // 2, 64 * (h % 2)
            ch = 32 * (h // 2)
            hsl = slice(b64, b64 + 64)
            for g in range(G):
                gsl = slice(g * W, (g + 1) * W)
                psl = slice((g - 1) * W, g * W)
                pss = psum.tile([128, 384], f32, tag="big", bufs=2)
                if g > 0:
                    nc.tensor.matmul(pss[:, 0:128], kT[t][hsl, psl], qT[t][hsl, gsl],
                                     start=True, stop=True)
                nc.tensor.matmul(pss[:, 128:256], kT[t][hsl, gsl], qT[t][hsl, gsl],
                                 start=True, stop=True)
                nc.tensor.matmul(pss[0:32, 256:384], rfkbT16[hsl, ch:ch + 32],
                                 qT[t][hsl, gsl], start=True, stop=True)
                expd = wkp.tile([128, 256], f16, tag="expd", bufs=3)
                if g > 0:
                    nc.scalar.activation(expd[:], pss[:, 0:256], AF.Exp)
                else:
                    nc.scalar.activation(expd[:, 128:256], pss[:, 128:256], AF.Exp)
                nc.vector.tensor_tensor(out=expd[:, 128:256], in0=expd[:, 128:256],
                                        in1=mask_sb[:], op=ALU.mult)
                expr = wkp.tile([32, 128], f16, tag="expr", bufs=3)
                nc.scalar.activation(expr[:], pss[0:32, 256:384], AF.Exp,
                                     bias=bias_all[:, h, g:g + 1])
                pso = psum.tile([D + 1, 128], f32, tag="small", bufs=4)
                if g > 0:
                    nc.tensor.matmul(pso[:], v_aug[:, g - 1, h, :], expd[:, 0:128],
                                     start=True, stop=False)
                nc.tensor.matmul(pso[:], v_aug[:, g, h, :], expd[:, 128:256],
                                 start=(g == 0), stop=False)
                nc.tensor.matmul(pso[:], rfa_aug[:, h, :], expr[:],
                                 start=False, stop=True)
                rr16 = wkp.tile([65, 128], f16, tag="rr16", bufs=3)
                with nc.allow_low_precision("fp16 softmax denominators"):
                    nc.vector.reciprocal(rr16[64:65, :], pso[D:D + 1, :])
                psb = psum.tile([D, 128], f32, tag="small", bufs=4)
                nc.tensor.matmul(psb[:], ones_r64[64:65, 0:D], rr16[64:65, :],
                                 start=True, stop=True)
                o16 = wkp.tile([D, 128], f16, tag="o16", bufs=3)
                nc.scalar.copy(o16[:], pso[0:D, :])
                stage = wkp.tile([D, 128], f16, tag="stage", bufs=3)
                nc.vector.tensor_tensor(out=stage[:], in0=o16[:], in1=psb[:], op=ALU.mult)
                nc.sync.dma_start(aot[h * D:(h + 1) * D, gsl], stage[:])

        # ---------------- phase 4: output projection (partial) ----------------
        for ns in range(8):
            nsl = slice(ns * 512, (ns + 1) * 512)
            aosb = wkp.tile([128, 4, 512], f16, tag="aosb")
            nc.sync.dma_start(aosb[:], aot.rearrange("(k p) n -> p k n", p=128)[:, :, nsl])
            for e in range(8):
                ps = psum.tile([128, 512], f32, tag="big", bufs=2)
                for k in range(4):
                    nc.tensor.matmul(ps[:], wot_sb[:, k, e * 128:(e + 1) * 128],
                                     aosb[:, k, :], start=(k == 0), stop=(k == 3))
                stg = wkp.tile([128, 512], f16, tag="stg", bufs=3)
                nc.scalar.copy(stg[:], ps[:])
                nc.sync.dma_start(outT[e * 128:(e + 1) * 128, nsl], stg[:])

    nc.compile()
    return nc


def _host_prep(inputs):
    q32 = np.asarray(inputs["query"], dtype=np.float32)
    Wq, bq = np.asarray(inputs["Wq"], np.float32), np.asarray(inputs["bq"], np.float32)
    Wk, bk = np.asarray(inputs["Wk"], np.float32), np.asarray(inputs["bk"], np.float32)
    Wv, bv = np.asarray(inputs["Wv"], np.float32), np.asarray(inputs["bv"], np.float32)
    Wo = np.asarray(inputs["Wo"], np.float32)
    f16 = np.float16

    j = np.arange(128)
    mask01 = (j[:, None] <= j[None, :]).astype(f16)          # [j_rel, i]
    cc, gg = np.arange(32)[:, None], np.arange(32)[None, :]
    maskbias = np.where(cc < gg, 0.0, NEG).astype(np.float32)
    ident = np.eye(128)

    common = {
        "mask01": mask01,
        "maskbias": maskbias,
        "ident16": ident.astype(f16),
        "ident32": ident.astype(np.float32),
        "neghalf": np.full((128, 1), -SCALE / 2, f16),
        "ones16": np.ones((1, 128), f16),
        "epscol": np.full((128, 1), 1e-5, np.float32),
        "onesv": np.ones((128, 32 * 8), f16),
        "mubq_bc": np.broadcast_to(np.asarray(inputs["mu_q_b"], np.float32), (128, D)).copy(),
        "mubk_bc": np.broadcast_to(np.asarray(inputs["mu_k_b"], np.float32), (128, D)).copy(),
        "muqw": (np.asarray(inputs["mu_q_w"], np.float32).T / 128.0).astype(f16),
        "mukw": (np.asarray(inputs["mu_k_w"], np.float32).T / 128.0).astype(f16),
        "lnconst": np.concatenate([
            np.broadcast_to(np.asarray(inputs["mu_q_g"], np.float32), (128, D)),
            np.broadcast_to(np.asarray(inputs["mu_q_be"], np.float32), (128, D)),
            np.broadcast_to(np.asarray(inputs["mu_k_g"], np.float32), (128, D)),
            np.broadcast_to(np.asarray(inputs["mu_k_be"], np.float32), (128, D)),
        ], axis=1).copy(),
    }

    per_hg = []
    for hg in range(2):
        hs = slice(hg * 512, (hg + 1) * 512)
        wtc = np.concatenate([Wq[hs].T, Wk[hs].T, Wv[hs].T], axis=1)
        bqkc = np.concatenate([bq[hs] * SCALE, bk[hs]]).reshape(8, 128).T
        per_hg.append({
            "wt": np.ascontiguousarray(wtc).astype(f16),
            "bqk": np.ascontiguousarray(bqkc).astype(np.float32),
            "bvrow": bv[hs].reshape(1, 512).astype(f16),
            "wot": np.ascontiguousarray(Wo[:, hs].T).astype(f16),
        })

    in_maps = []
    for core in range(8):
        b, hg = core // 2, core % 2
        m = dict(common)
        m.update(per_hg[hg])
        m["xt"] = np.ascontiguousarray(q32[:, b, :].T).astype(f16)
        in_maps.append(m)
    return in_maps


def kernel(**inputs):
    if "nc" not in _CACHED:
        _CACHED["nc"] = _build_nc()
    nc = _CACHED["nc"]
    in_maps = _host_prep(inputs)
    run_kwargs = _CACHED.get("run_kwargs", {})
    res = run_bass_kernel_spmd(nc, in_maps, core_ids=list(range(8)), **run_kwargs)
    _CACHED["last_result"] = res

    bo = np.asarray(inputs["bo"], np.float32)
    out = np.empty((N, B, E), np.float32)
    for b in range(B):
        acc = res.results[2 * b]["outT"].astype(np.float32) \
            + res.results[2 * b + 1]["outT"].astype(np.float32)
        out[:, b, :] = acc.T + bo
    return out


# revision 9
# speedup vs baseline: 1.5495x; 1.5495x over previous
"""Trainium2 Bass kernel for CausalEVAttention (sparse_attention).

Sharding: 8 cores = 4 batches x 2 head-groups (8 heads each).
Each core computes QKV projections (fp16 matmuls), windowed local causal
attention + EVA random-feature chunk branch, and a partial output
projection over its head group.  Host sums the two head-group partials
per batch and adds the output bias.

All heavy matmuls run in fp16 (inputs pre-cast on host); accumulation is
fp32 in PSUM.  Softmax runs without max-subtraction (logit magnitudes are
small); masked lanes use -1e9 biases.  The beta (within-chunk) softmax is
folded into the joint softmax: rfa value blocks stay unnormalized and the
pseudo-key logits get a -ln(sum) per-partition bias.
"""

import numpy as np

import concourse.bass as bass
import concourse.mybir as mybir
import concourse.tile as tile
from concourse import bacc
from concourse.bass_utils import run_bass_kernel_spmd

dt = mybir.dt
AF = mybir.ActivationFunctionType
ALU = mybir.AluOpType

N, B, E, H = 4096, 4, 1024, 16
D = 64                # head dim
HPC = 8               # heads per core
G = 32                # windows (128 queries each)
C = 32                # rf chunks (128 keys each)
W = 128               # window size
SCALE = D ** -0.5     # 0.125
NEG = -1e9

_CACHED = {}


def _build_nc():
    nc = bacc.Bacc("TRN2", target_bir_lowering=False, debug=False, num_devices=8)

    f16, f32 = dt.float16, dt.float32
    inp = lambda name, shape, d: nc.dram_tensor(name, shape, d, kind="ExternalInput").ap()

    xt = inp("xt", [E, N], f16)               # query[:, b, :].T
    wt = inp("wt", [E, 3 * 512], f16)         # [WqT | WkT | WvT] head-group slice
    bqk = inp("bqk", [128, 8], f32)           # packed (bq*0.125 | bk) per m-tile
    bvrow = inp("bvrow", [1, 512], f16)
    wot = inp("wot", [512, E], f16)           # Wo[:, hs].T
    muqw = inp("muqw", [D, D], f16)           # mu_q_w.T / 128
    mukw = inp("mukw", [D, D], f16)
    mubq_bc = inp("mubq_bc", [128, D], f32)   # mu_q_b broadcast
    mubk_bc = inp("mubk_bc", [128, D], f32)
    lnconst = inp("lnconst", [128, 4 * D], f32)  # [gq | beq | gk | bek] broadcasts
    mask01 = inp("mask01", [128, 128], f16)   # causal keep-mask (S^T diag block)
    maskbias = inp("maskbias", [32, 32], f32)  # chunk-visibility bias [c, g]
    ident16 = inp("ident16", [128, 128], f16)
    ident32 = inp("ident32", [128, 128], f32)
    neghalf = inp("neghalf", [128, 1], f16)   # -scale/2
    ones16 = inp("ones16", [1, 128], f16)
    epscol = inp("epscol", [128, 1], f32)
    onesv = inp("onesv", [128, C * HPC], f16)  # ones for v_aug 65th column

    outT = nc.dram_tensor("outT", [E, N], f16, kind="ExternalOutput").ap()

    from contextlib import ExitStack
    with tile.TileContext(nc) as tc, ExitStack() as stk:
        cpool = stk.enter_context(tc.tile_pool(name="consts", bufs=1))
        bigp = stk.enter_context(tc.tile_pool(name="bigs", bufs=1))
        wkp = stk.enter_context(tc.tile_pool(name="work", bufs=2))
        psum = stk.enter_context(tc.tile_pool(name="ps", bufs=1, space="PSUM"))

        # ---------------- constants / weights ----------------
        wt_sb = cpool.tile([128, 8, 3 * 512], f16)
        nc.sync.dma_start(wt_sb[:], wt.rearrange("(k p) m -> p k m", p=128))
        wot_sb = cpool.tile([128, 4, E], f16)
        nc.sync.dma_start(wot_sb[:], wot.rearrange("(k p) m -> p k m", p=128))
        bqk_sb = cpool.tile([128, 8], f32)
        nc.sync.dma_start(bqk_sb[:], bqk)
        bvrow_sb = cpool.tile([1, 512], f16)
        nc.sync.dma_start(bvrow_sb[:], bvrow)
        muqw_sb = cpool.tile([128, D], f16)   # duplicated across halves
        nc.sync.dma_start(muqw_sb[0:64, :], muqw)
        nc.sync.dma_start(muqw_sb[64:128, :], muqw)
        mukw_sb = cpool.tile([128, D], f16)
        nc.sync.dma_start(mukw_sb[0:64, :], mukw)
        nc.sync.dma_start(mukw_sb[64:128, :], mukw)
        mubq_sb = cpool.tile([128, D], f32)
        nc.sync.dma_start(mubq_sb[:], mubq_bc)
        mubk_sb = cpool.tile([128, D], f32)
        nc.sync.dma_start(mubk_sb[:], mubk_bc)
        lnc_sb = cpool.tile([128, 4 * D], f32)
        nc.sync.dma_start(lnc_sb[:], lnconst)
        mask_sb = cpool.tile([128, 128], f16)
        nc.sync.dma_start(mask_sb[:], mask01)
        mbias_sb = cpool.tile([32, 32], f32)
        nc.sync.dma_start(mbias_sb[:], maskbias)
        id16_sb = cpool.tile([128, 128], f16)
        nc.sync.dma_start(id16_sb[:], ident16)
        id32_sb = cpool.tile([128, 128], f32)
        nc.sync.dma_start(id32_sb[:], ident32)
        ngh_sb = cpool.tile([128, 1], f16)
        nc.sync.dma_start(ngh_sb[:], neghalf)
        ones_sb = cpool.tile([1, 128], f16)
        nc.sync.dma_start(ones_sb[:], ones16)
        eps_sb = cpool.tile([128, 1], f32)
        nc.sync.dma_start(eps_sb[:], epscol)

        # ---------------- big persistent tensors ----------------
        qT = [bigp.tile([128, N], f16, tag=f"qT{t}", name=f"qT{t}") for t in range(4)]
        kT = [bigp.tile([128, N], f16, tag=f"kT{t}", name=f"kT{t}") for t in range(4)]
        kT2 = [bigp.tile([128, N], f16, tag=f"kT2{t}", name=f"kT2{t}") for t in range(4)]
        v_aug = bigp.tile([128, C, HPC, D + 1], f16)
        nc.sync.dma_start(v_aug[:, :, :, D], onesv)

        # ---------------- phase 1: QKV projections ----------------
        for ns in range(8):
            nsl = slice(ns * 512, (ns + 1) * 512)
            xs = wkp.tile([128, 8, 512], f16, tag="xs")
            nc.sync.dma_start(xs[:], xt.rearrange("(k p) n -> p k n", p=128)[:, :, nsl])
            for m in range(8):
                ps = psum.tile([128, 512], f32, tag="big", bufs=2)
                for k in range(8):
                    nc.tensor.matmul(ps[:], wt_sb[:, k, m * 128:(m + 1) * 128],
                                     xs[:, k, :], start=(k == 0), stop=(k == 7))
                if m < 4:
                    nc.scalar.activation(qT[m][:, nsl], ps[:], AF.Identity,
                                         bias=bqk_sb[:, m:m + 1], scale=SCALE)
                else:
                    nc.scalar.activation(kT[m - 4][:, nsl], ps[:], AF.Identity,
                                         bias=bqk_sb[:, m:m + 1], scale=1.0)
            for nb in range(4):
                g = ns * 4 + nb
                ps = psum.tile([128, 512], f32, tag="big", bufs=2)
                for k in range(8):
                    nc.tensor.matmul(ps[:], xs[:, k, nb * 128:(nb + 1) * 128],
                                     wt_sb[:, k, 1024:1536], start=(k == 0), stop=False)
                nc.tensor.matmul(ps[:], ones_sb[0:1, :], bvrow_sb[0:1, :],
                                 start=False, stop=True)
                nc.scalar.copy(v_aug[:, g, :, 0:D], ps[:].rearrange("p (h d) -> p h d", d=D))

        # ---------------- phase 2: RFA statistics ----------------
        meansQ = wkp.tile([128, 4, C], f32, tag="meansQ", bufs=1)
        meansK = wkp.tile([128, 4, C], f32, tag="meansK", bufs=1)
        for t in range(4):
            nc.vector.tensor_reduce(out=meansQ[:, t, :],
                                    in_=qT[t][:].rearrange("p (c w) -> p c w", w=W),
                                    op=ALU.add, axis=mybir.AxisListType.X)
            nc.vector.tensor_reduce(out=meansK[:, t, :],
                                    in_=kT[t][:].rearrange("p (c w) -> p c w", w=W),
                                    op=ALU.add, axis=mybir.AxisListType.X)
        meansQ16 = wkp.tile([128, 4, C], f16, tag="mQ16", bufs=1)
        meansK16 = wkp.tile([128, 4, C], f16, tag="mK16", bufs=1)
        nc.scalar.copy(meansQ16[:], meansQ[:])
        nc.scalar.copy(meansK16[:], meansK[:])

        # per-head linear + layernorm (both sides), then mu = qbar + kbar
        mu_pack = wkp.tile([128, 128], f32, tag="mu_pack", bufs=1)
        rfk_pack = wkp.tile([128, 128], f32, tag="rfk_pack", bufs=1)
        for h in range(HPC):
            t, b64 = h // 2, 64 * (h % 2)
            jr, jc = h // 2, h % 2
            bars = []
            for side in range(2):  # 0 = q, 1 = k
                mw = muqw_sb if side == 0 else mukw_sb
                mean16 = meansQ16 if side == 0 else meansK16
                mub = mubq_sb if side == 0 else mubk_sb
                gofs = 0 if side == 0 else 2 * D
                psl = psum.tile([32, D], f32, tag="small", bufs=4)
                nc.tensor.matmul(psl[:], mean16[b64:b64 + 64, t, :],
                                 mw[b64:b64 + 64, :], start=True, stop=True)
                x = wkp.tile([32, D], f32, tag=f"lnx{side}", bufs=2)
                nc.vector.tensor_tensor(out=x[:], in0=psl[:], in1=mub[0:32, :], op=ALU.add)
                mn = wkp.tile([32, 1], f32, tag=f"lnm{side}", bufs=2)
                nc.vector.tensor_reduce(out=mn[:], in_=x[:], op=ALU.add,
                                        axis=mybir.AxisListType.X)
                nc.vector.tensor_scalar_mul(mn[:], mn[:], 1.0 / D)
                nc.vector.tensor_scalar(out=x[:], in0=x[:], scalar1=mn[:],
                                        scalar2=None, op0=ALU.subtract)
                junk = wkp.tile([32, D], f32, tag="junk", bufs=2)
                var = wkp.tile([32, 1], f32, tag=f"lnv{side}", bufs=2)
                nc.scalar.activation(junk[:], x[:], AF.Square, scale=float(D ** -0.5),
                                     accum_out=var[:])
                nc.scalar.activation(var[:], var[:], AF.Sqrt, bias=eps_sb[0:32, :])
                nc.vector.reciprocal(var[:], var[:])
                nc.vector.tensor_scalar_mul(x[:], x[:], var[:])
                bar = wkp.tile([32, D], f32, tag=f"bar{side}", bufs=2)
                nc.vector.scalar_tensor_tensor(out=bar[:], in0=x[:], scalar=1.0,
                                               in1=lnc_sb[0:32, gofs:gofs + D],
                                               op0=ALU.mult, op1=ALU.mult)
                nc.vector.tensor_tensor(out=bar[:], in0=bar[:],
                                        in1=lnc_sb[0:32, gofs + D:gofs + 2 * D], op=ALU.add)
                bars.append(bar)
            mu_h = wkp.tile([32, D], f32, tag="mu_h", bufs=2)
            nc.vector.tensor_tensor(out=mu_h[:], in0=bars[0][:], in1=bars[1][:], op=ALU.add)
            nc.sync.dma_start(mu_pack[32 * jr:32 * jr + 32, 64 * jc:64 * jc + 64], mu_h[:])
            nc.sync.dma_start(rfk_pack[32 * jr:32 * jr + 32, 64 * jc:64 * jc + 64], bars[1][:])

        # transpose packs -> muT16 (scaled), rfkbT16
        muT16 = wkp.tile([128, 128], f16, tag="muT16", bufs=1)
        rfkbT16 = wkp.tile([128, 128], f16, tag="rfkbT16", bufs=1)
        pst = psum.tile([128, 128], f32, tag="smallb", bufs=2)
        nc.tensor.transpose(pst[:], mu_pack[:], id32_sb[:])
        nc.scalar.activation(muT16[:], pst[:], AF.Copy, scale=SCALE)
        pst2 = psum.tile([128, 128], f32, tag="smallb", bufs=2)
        nc.tensor.transpose(pst2[:], rfk_pack[:], id32_sb[:])
        nc.scalar.copy(rfkbT16[:], pst2[:])

        # kT squared (for the -|k|^2/2 term)
        for t in range(4):
            nc.vector.tensor_tensor(out=kT2[t][:], in0=kT[t][:], in1=kT[t][:], op=ALU.mult)

        # per-head chunk logits, exp, unnormalized U, bias table
        rfa_aug = wkp.tile([32, HPC, D + 1], f16, tag="rfa_aug", bufs=1)
        bias_all = wkp.tile([32, HPC, G], f32, tag="bias_all", bufs=1)
        for h in range(HPC):
            t, b64 = h // 2, 64 * (h % 2)
            ch = 32 * (h // 2)
            hsl = slice(b64, b64 + 64)
            pslp = psum.tile([128, C], f32, tag="small", bufs=4)
            for c in range(C):
                csl = slice(c * W, (c + 1) * W)
                nc.tensor.matmul(pslp[:, c:c + 1], kT[t][hsl, csl],
                                 muT16[hsl, ch + c:ch + c + 1], start=True, stop=False)
                nc.tensor.matmul(pslp[:, c:c + 1], kT2[t][hsl, csl],
                                 ngh_sb[hsl, :], start=False, stop=True)
            explp = wkp.tile([128, C], f16, tag="explp", bufs=2)
            nc.scalar.activation(explp[:], pslp[:], AF.Exp)
            psu = psum.tile([D + 1, C], f32, tag="small", bufs=4)
            for c in range(C):
                nc.tensor.matmul(psu[:, c:c + 1], v_aug[:, c, h, :],
                                 explp[:, c:c + 1], start=True, stop=True)
            u16 = wkp.tile([D + 1, C], f16, tag="u16", bufs=2)
            nc.scalar.activation(u16[:], psu[:], AF.Copy, scale=1.0 / 16)
            psut = psum.tile([C, D + 1], f16, tag="smallb", bufs=2)
            nc.tensor.transpose(psut[:], u16[:], id16_sb[0:D + 1, 0:D + 1])
            nc.scalar.copy(rfa_aug[:, h, :], psut[:])
            lns = wkp.tile([32, 1], f32, tag="lns", bufs=2)
            nc.scalar.activation(lns[:], rfa_aug[:, h, D:D + 1], AF.Ln)
            nc.vector.tensor_scalar(out=bias_all[:, h, :], in0=mbias_sb[:],
                                    scalar1=lns[:], scalar2=None, op0=ALU.subtract)

        # ---------------- phase 3: windowed attention ----------------
        # aon[t][:, w, :] holds attention output rows for window w in
        # natural (i, hd) layout; reuses the kT2 slots (tag match).
        aon = [bigp.tile([128, 8, 512], f16, tag=f"kT2{t}", name=f"aon{t}")
               for t in range(4)]
        for h in range(HPC):
            t, b64 = h // 2, 64 * (h % 2)
            ch = 32 * (h // 2)
            hsl = slice(b64, b64 + 64)
            for g0 in range(0, G, 2):
                g1 = g0 + 1
                q2 = qT[t][hsl, g0 * W:(g0 + 2) * W]          # queries of the pair
                pss = psum.tile([128, 512], f32, tag="big", bufs=2)
                if g0 > 0:
                    nc.tensor.matmul(pss[:, 0:128],
                                     kT[t][hsl, (g0 - 1) * W:g0 * W],
                                     qT[t][hsl, g0 * W:(g0 + 1) * W],
                                     start=True, stop=True)
                nc.tensor.matmul(pss[:, 128:384], kT[t][hsl, g0 * W:(g0 + 1) * W],
                                 q2, start=True, stop=True)
                nc.tensor.matmul(pss[:, 384:512], kT[t][hsl, g1 * W:(g1 + 1) * W],
                                 qT[t][hsl, g1 * W:(g1 + 1) * W],
                                 start=True, stop=True)
                psr = psum.tile([32, 256], f32, tag="small", bufs=4)
                nc.tensor.matmul(psr[:], rfkbT16[hsl, ch:ch + 32], q2,
                                 start=True, stop=True)
                expd = wkp.tile([128, 512], f16, tag="expd", bufs=3)
                if g0 > 0:
                    nc.scalar.activation(expd[:], pss[:], AF.Exp)
                else:
                    nc.scalar.activation(expd[:, 128:512], pss[:, 128:512], AF.Exp)
                nc.vector.tensor_tensor(out=expd[:, 128:256], in0=expd[:, 128:256],
                                        in1=mask_sb[:], op=ALU.mult)
                nc.vector.tensor_tensor(out=expd[:, 384:512], in0=expd[:, 384:512],
                                        in1=mask_sb[:], op=ALU.mult)
                expr = wkp.tile([32, 256], f16, tag="expr", bufs=3)
                nc.scalar.activation(expr[:, 0:128], psr[:, 0:128], AF.Exp,
                                     bias=bias_all[:, h, g0:g0 + 1])
                nc.scalar.activation(expr[:, 128:256], psr[:, 128:256], AF.Exp,
                                     bias=bias_all[:, h, g1:g1 + 1])
                for w in (g0, g1):
                    dbase = 128 + 256 * (w - g0)   # diag block columns in expd
                    pso = psum.tile([128, D + 1], f32, tag="small", bufs=4)
                    if w > 0:
                        nc.tensor.matmul(pso[:], expd[:, dbase - 128:dbase],
                                         v_aug[:, w - 1, h, :], start=True, stop=False)
                    nc.tensor.matmul(pso[:], expd[:, dbase:dbase + 128],
                                     v_aug[:, w, h, :], start=(w == 0), stop=False)
                    nc.tensor.matmul(pso[:], expr[:, (w - g0) * 128:(w - g0 + 1) * 128],
                                     rfa_aug[:, h, :], start=False, stop=True)
                    rr = wkp.tile([128, 1], f32, tag="rr", bufs=3)
                    nc.vector.reciprocal(rr[:], pso[:, D:D + 1])
                    nc.scalar.activation(aon[w // 8][:, w % 8, h * D:(h + 1) * D],
                                         pso[:, 0:D], AF.Copy, scale=rr[:])

        # ---------------- phase 4: output projection (partial) ----------------
        for ns in range(8):
            nsl = slice(ns * 512, (ns + 1) * 512)
            aotT = wkp.tile([128, 4, 512], f16, tag="aotT", bufs=2)
            for te in range(4):       # hd tile
                for wi in range(4):   # window within n-slice
                    w = ns * 4 + wi
                    pstr = psum.tile([128, 128], f16, tag="smallb", bufs=2)
                    nc.tensor.transpose(pstr[:], aon[w // 8][:, w % 8,
                                                            te * 128:(te + 1) * 128],
                                        id16_sb[:])
                    nc.scalar.copy(aotT[:, te, wi * 128:(wi + 1) * 128], pstr[:])
            for e in range(8):
                ps = psum.tile([128, 512], f32, tag="big", bufs=2)
                for k in range(4):
                    nc.tensor.matmul(ps[:], wot_sb[:, k, e * 128:(e + 1) * 128],
                                     aotT[:, k, :], start=(k == 0), stop=(k == 3))
                stg = wkp.tile([128, 512], f16, tag="stg", bufs=3)
                nc.scalar.copy(stg[:], ps[:])
                nc.sync.dma_start(outT[e * 128:(e + 1) * 128, nsl], stg[:])

    nc.compile()
    return nc


def _host_prep(inputs):
    q32 = np.asarray(inputs["query"], dtype=np.float32)
    Wq, bq = np.asarray(inputs["Wq"], np.float32), np.asarray(inputs["bq"], np.float32)
    Wk, bk = np.asarray(inputs["Wk"], np.float32), np.asarray(inputs["bk"], np.float32)
    Wv, bv = np.asarray(inputs["Wv"], np.float32), np.asarray(inputs["bv"], np.float32)
    Wo = np.asarray(inputs["Wo"], np.float32)
    f16 = np.float16

    j = np.arange(128)
    mask01 = (j[:, None] <= j[None, :]).astype(f16)          # [j_rel, i]
    cc, gg = np.arange(32)[:, None], np.arange(32)[None, :]
    maskbias = np.where(cc < gg, 0.0, NEG).astype(np.float32)
    ident = np.eye(128)

    common = {
        "mask01": mask01,
        "maskbias": maskbias,
        "ident16": ident.astype(f16),
        "ident32": ident.astype(np.float32),
        "neghalf": np.full((128, 1), -SCALE / 2, f16),
        "ones16": np.ones((1, 128), f16),
        "epscol": np.full((128, 1), 1e-5, np.float32),
        "onesv": np.ones((128, 32 * 8), f16),
        "mubq_bc": np.broadcast_to(np.asarray(inputs["mu_q_b"], np.float32), (128, D)).copy(),
        "mubk_bc": np.broadcast_to(np.asarray(inputs["mu_k_b"], np.float32), (128, D)).copy(),
        "muqw": (np.asarray(inputs["mu_q_w"], np.float32).T / 128.0).astype(f16),
        "mukw": (np.asarray(inputs["mu_k_w"], np.float32).T / 128.0).astype(f16),
        "lnconst": np.concatenate([
            np.broadcast_to(np.asarray(inputs["mu_q_g"], np.float32), (128, D)),
            np.broadcast_to(np.asarray(inputs["mu_q_be"], np.float32), (128, D)),
            np.broadcast_to(np.asarray(inputs["mu_k_g"], np.float32), (128, D)),
            np.broadcast_to(np.asarray(inputs["mu_k_be"], np.float32), (128, D)),
        ], axis=1).copy(),
    }

    per_hg = []
    for hg in range(2):
        hs = slice(hg * 512, (hg + 1) * 512)
        wtc = np.concatenate([Wq[hs].T, Wk[hs].T, Wv[hs].T], axis=1)
        bqkc = np.concatenate([bq[hs] * SCALE, bk[hs]]).reshape(8, 128).T
        per_hg.append({
            "wt": np.ascontiguousarray(wtc).astype(f16),
            "bqk": np.ascontiguousarray(bqkc).astype(np.float32),
            "bvrow": bv[hs].reshape(1, 512).astype(f16),
            "wot": np.ascontiguousarray(Wo[:, hs].T).astype(f16),
        })

    in_maps = []
    for core in range(8):
        b, hg = core // 2, core % 2
        m = dict(common)
        m.update(per_hg[hg])
        m["xt"] = np.ascontiguousarray(q32[:, b, :].T).astype(f16)
        in_maps.append(m)
    return in_maps


def kernel(**inputs):
    if "nc" not in _CACHED:
        _CACHED["nc"] = _build_nc()
    nc = _CACHED["nc"]
    in_maps = _host_prep(inputs)
    run_kwargs = _CACHED.get("run_kwargs", {})
    res = run_bass_kernel_spmd(nc, in_maps, core_ids=list(range(8)), **run_kwargs)
    _CACHED["last_result"] = res

    bo = np.asarray(inputs["bo"], np.float32)
    out = np.empty((N, B, E), np.float32)
    for b in range(B):
        acc = res.results[2 * b]["outT"].astype(np.float32) \
            + res.results[2 * b + 1]["outT"].astype(np.float32)
        out[:, b, :] = acc.T + bo
    return out


# revision 12
# speedup vs baseline: 1.6754x; 1.0812x over previous
"""Trainium2 Bass kernel for CausalEVAttention (sparse_attention).

Sharding: 8 cores = 4 batches x 2 head-groups (8 heads each).
Each core computes QKV projections (fp16 matmuls), windowed local causal
attention + EVA random-feature chunk branch, and a partial output
projection over its head group.  Host sums the two head-group partials
per batch and adds the output bias.

All heavy matmuls run in fp16 (inputs pre-cast on host); accumulation is
fp32 in PSUM.  Softmax runs without max-subtraction (logit magnitudes are
small); masked lanes use -1e9 biases.  The beta (within-chunk) softmax is
folded into the joint softmax: rfa value blocks stay unnormalized and the
pseudo-key logits get a -ln(sum) per-partition bias.
"""

import numpy as np

import concourse.bass as bass
import concourse.mybir as mybir
import concourse.tile as tile
from concourse import bacc
from concourse.bass_utils import run_bass_kernel_spmd

dt = mybir.dt
AF = mybir.ActivationFunctionType
ALU = mybir.AluOpType

N, B, E, H = 4096, 4, 1024, 16
D = 64                # head dim
HPC = 8               # heads per core
G = 32                # windows (128 queries each)
C = 32                # rf chunks (128 keys each)
W = 128               # window size
SCALE = D ** -0.5     # 0.125
NEG = -1e9

_CACHED = {}


def _build_nc():
    nc = bacc.Bacc("TRN2", target_bir_lowering=False, debug=False, num_devices=8)

    f16, f32 = dt.float16, dt.float32
    inp = lambda name, shape, d: nc.dram_tensor(name, shape, d, kind="ExternalInput").ap()

    xt = inp("xt", [E, N], f16)               # query[:, b, :].T
    wt = inp("wt", [E, 3 * 512], f16)         # [WqT | WkT | WvT] head-group slice
    bqk = inp("bqk", [128, 8], f32)           # packed (bq*0.125 | bk) per m-tile
    bvrow = inp("bvrow", [1, 512], f16)
    wot = inp("wot", [512, E], f16)           # Wo[:, hs].T
    muqw = inp("muqw", [D, D], f16)           # mu_q_w.T / 128
    mukw = inp("mukw", [D, D], f16)
    mubq_bc = inp("mubq_bc", [128, D], f32)   # mu_q_b broadcast
    mubk_bc = inp("mubk_bc", [128, D], f32)
    lnconst = inp("lnconst", [128, 4 * D], f32)  # [gq | beq | gk | bek] broadcasts
    mask01 = inp("mask01", [128, 128], f16)   # causal keep-mask (S^T diag block)
    ident16 = inp("ident16", [128, 128], f16)
    ident32 = inp("ident32", [128, 128], f32)
    neghalf = inp("neghalf", [128, 1], f16)   # -scale/2
    ones16 = inp("ones16", [1, 128], f16)
    epscol = inp("epscol", [128, 1], f32)
    onesv = inp("onesv", [128, C * HPC], f16)  # ones for v_aug 65th column

    outT = nc.dram_tensor("outT", [E, N], f16, kind="ExternalOutput").ap()

    from contextlib import ExitStack
    with tile.TileContext(nc) as tc, ExitStack() as stk:
        cpool = stk.enter_context(tc.tile_pool(name="consts", bufs=1))
        bigp = stk.enter_context(tc.tile_pool(name="bigs", bufs=1))
        wkp = stk.enter_context(tc.tile_pool(name="work", bufs=2))
        psum = stk.enter_context(tc.tile_pool(name="ps", bufs=1, space="PSUM"))

        # ---------------- constants / weights ----------------
        wt_sb = cpool.tile([128, 8, 3 * 512], f16)
        nc.sync.dma_start(wt_sb[:], wt.rearrange("(k p) m -> p k m", p=128))
        wot_sb = cpool.tile([128, 4, E], f16)
        nc.sync.dma_start(wot_sb[:], wot.rearrange("(k p) m -> p k m", p=128))
        bqk_sb = cpool.tile([128, 8], f32)
        nc.sync.dma_start(bqk_sb[:], bqk)
        bvrow_sb = cpool.tile([1, 512], f16)
        nc.sync.dma_start(bvrow_sb[:], bvrow)
        muqw_sb = cpool.tile([128, D], f16)   # duplicated across halves
        nc.sync.dma_start(muqw_sb[0:64, :], muqw)
        nc.sync.dma_start(muqw_sb[64:128, :], muqw)
        mukw_sb = cpool.tile([128, D], f16)
        nc.sync.dma_start(mukw_sb[0:64, :], mukw)
        nc.sync.dma_start(mukw_sb[64:128, :], mukw)
        mubq_sb = cpool.tile([128, D], f32)
        nc.sync.dma_start(mubq_sb[:], mubq_bc)
        mubk_sb = cpool.tile([128, D], f32)
        nc.sync.dma_start(mubk_sb[:], mubk_bc)
        lnc_sb = cpool.tile([128, 4 * D], f32)
        nc.sync.dma_start(lnc_sb[:], lnconst)
        mask_sb = cpool.tile([128, 128], f16)
        nc.sync.dma_start(mask_sb[:], mask01)
        id16_sb = cpool.tile([128, 128], f16)
        nc.sync.dma_start(id16_sb[:], ident16)
        id32_sb = cpool.tile([128, 128], f32)
        nc.sync.dma_start(id32_sb[:], ident32)
        ngh_sb = cpool.tile([128, 1], f16)
        nc.sync.dma_start(ngh_sb[:], neghalf)
        ones_sb = cpool.tile([1, 128], f16)
        nc.sync.dma_start(ones_sb[:], ones16)
        eps_sb = cpool.tile([128, 1], f32)
        nc.sync.dma_start(eps_sb[:], epscol)

        # ---------------- big persistent tensors ----------------
        qT = [bigp.tile([128, N], f16, tag=f"qT{t}", name=f"qT{t}") for t in range(4)]
        kT = [bigp.tile([128, N], f16, tag=f"kT{t}", name=f"kT{t}") for t in range(4)]
        kT2 = [bigp.tile([128, N], f16, tag=f"kT2{t}", name=f"kT2{t}") for t in range(4)]
        v_aug = bigp.tile([128, C, HPC, D + 1], f16)
        nc.sync.dma_start(v_aug[:, :, :, D], onesv)

        # ---------------- phase 1: QKV projections ----------------
        for ns in range(8):
            nsl = slice(ns * 512, (ns + 1) * 512)
            xs = wkp.tile([128, 8, 512], f16, tag="xs")
            nc.sync.dma_start(xs[:], xt.rearrange("(k p) n -> p k n", p=128)[:, :, nsl])
            for m in range(8):
                ps = psum.tile([128, 512], f32, tag="big", bufs=2)
                for k in range(8):
                    nc.tensor.matmul(ps[:], wt_sb[:, k, m * 128:(m + 1) * 128],
                                     xs[:, k, :], start=(k == 0), stop=(k == 7))
                if m < 4:
                    nc.scalar.activation(qT[m][:, nsl], ps[:], AF.Identity,
                                         bias=bqk_sb[:, m:m + 1], scale=SCALE)
                else:
                    nc.scalar.activation(kT[m - 4][:, nsl], ps[:], AF.Identity,
                                         bias=bqk_sb[:, m:m + 1], scale=1.0)
            for nb in range(4):
                g = ns * 4 + nb
                ps = psum.tile([128, 512], f32, tag="big", bufs=2)
                for k in range(8):
                    nc.tensor.matmul(ps[:], xs[:, k, nb * 128:(nb + 1) * 128],
                                     wt_sb[:, k, 1024:1536], start=(k == 0), stop=False)
                nc.tensor.matmul(ps[:], ones_sb[0:1, :], bvrow_sb[0:1, :],
                                 start=False, stop=True)
                nc.vector.tensor_copy(v_aug[:, g, :, 0:D], ps[:].rearrange("p (h d) -> p h d", d=D))

        # ---------------- phase 2: RFA statistics ----------------
        meansQ = wkp.tile([128, 4, C], f32, tag="meansQ", bufs=1)
        meansK = wkp.tile([128, 4, C], f32, tag="meansK", bufs=1)
        for t in range(4):
            nc.vector.tensor_reduce(out=meansQ[:, t, :],
                                    in_=qT[t][:].rearrange("p (c w) -> p c w", w=W),
                                    op=ALU.add, axis=mybir.AxisListType.X)
            nc.vector.tensor_reduce(out=meansK[:, t, :],
                                    in_=kT[t][:].rearrange("p (c w) -> p c w", w=W),
                                    op=ALU.add, axis=mybir.AxisListType.X)
        meansQ16 = wkp.tile([128, 4, C], f16, tag="mQ16", bufs=1)
        meansK16 = wkp.tile([128, 4, C], f16, tag="mK16", bufs=1)
        nc.scalar.copy(meansQ16[:], meansQ[:])
        nc.scalar.copy(meansK16[:], meansK[:])

        # per-head linear + layernorm (both sides), then mu = qbar + kbar
        mu_pack = wkp.tile([128, 128], f32, tag="mu_pack", bufs=1)
        rfk_pack = wkp.tile([128, 128], f32, tag="rfk_pack", bufs=1)
        for h in range(HPC):
            t, b64 = h // 2, 64 * (h % 2)
            jr, jc = h // 2, h % 2
            bars = []
            for side in range(2):  # 0 = q, 1 = k
                mw = muqw_sb if side == 0 else mukw_sb
                mean16 = meansQ16 if side == 0 else meansK16
                mub = mubq_sb if side == 0 else mubk_sb
                gofs = 0 if side == 0 else 2 * D
                psl = psum.tile([32, D], f32, tag="small", bufs=4)
                nc.tensor.matmul(psl[:], mean16[b64:b64 + 64, t, :],
                                 mw[b64:b64 + 64, :], start=True, stop=True)
                x = wkp.tile([32, D], f32, tag=f"lnx{side}", bufs=2)
                nc.vector.tensor_tensor(out=x[:], in0=psl[:], in1=mub[0:32, :], op=ALU.add)
                mn = wkp.tile([32, 1], f32, tag=f"lnm{side}", bufs=2)
                nc.vector.tensor_reduce(out=mn[:], in_=x[:], op=ALU.add,
                                        axis=mybir.AxisListType.X)
                nc.vector.tensor_scalar_mul(mn[:], mn[:], 1.0 / D)
                nc.vector.tensor_scalar(out=x[:], in0=x[:], scalar1=mn[:],
                                        scalar2=None, op0=ALU.subtract)
                junk = wkp.tile([32, D], f32, tag="junk", bufs=2)
                var = wkp.tile([32, 1], f32, tag=f"lnv{side}", bufs=2)
                nc.scalar.activation(junk[:], x[:], AF.Square, scale=float(D ** -0.5),
                                     accum_out=var[:])
                nc.scalar.activation(var[:], var[:], AF.Sqrt, bias=eps_sb[0:32, :])
                nc.vector.reciprocal(var[:], var[:])
                nc.vector.tensor_scalar_mul(x[:], x[:], var[:])
                bar = wkp.tile([32, D], f32, tag=f"bar{side}", bufs=2)
                nc.vector.scalar_tensor_tensor(out=bar[:], in0=x[:], scalar=1.0,
                                               in1=lnc_sb[0:32, gofs:gofs + D],
                                               op0=ALU.mult, op1=ALU.mult)
                nc.vector.tensor_tensor(out=bar[:], in0=bar[:],
                                        in1=lnc_sb[0:32, gofs + D:gofs + 2 * D], op=ALU.add)
                bars.append(bar)
            mu_h = wkp.tile([32, D], f32, tag="mu_h", bufs=2)
            nc.vector.tensor_tensor(out=mu_h[:], in0=bars[0][:], in1=bars[1][:], op=ALU.add)
            nc.sync.dma_start(mu_pack[32 * jr:32 * jr + 32, 64 * jc:64 * jc + 64], mu_h[:])
            nc.sync.dma_start(rfk_pack[32 * jr:32 * jr + 32, 64 * jc:64 * jc + 64], bars[1][:])

        # transpose packs -> muT16 (scaled), rfkbT16
        muT16 = wkp.tile([128, 128], f16, tag="muT16", bufs=1)
        rfkbT16 = wkp.tile([128, 128], f16, tag="rfkbT16", bufs=1)
        pst = psum.tile([128, 128], f32, tag="smallb", bufs=2)
        nc.tensor.transpose(pst[:], mu_pack[:], id32_sb[:])
        nc.scalar.activation(muT16[:], pst[:], AF.Copy, scale=SCALE)
        pst2 = psum.tile([128, 128], f32, tag="smallb", bufs=2)
        nc.tensor.transpose(pst2[:], rfk_pack[:], id32_sb[:])
        nc.scalar.copy(rfkbT16[:], pst2[:])

        # kT squared (for the -|k|^2/2 term)
        for t in range(4):
            nc.vector.tensor_tensor(out=kT2[t][:], in0=kT[t][:], in1=kT[t][:], op=ALU.mult)

        # per-head chunk logits, exp, unnormalized U -> normalized rfa_aug
        # (rfa_aug rows are divided by the chunk softmax sum; masked chunks
        # are handled downstream by partial-K slicing, no bias needed)
        rfa_aug = wkp.tile([32, HPC, D + 1], f16, tag="rfa_aug", bufs=1)
        for h in range(HPC):
            t, b64 = h // 2, 64 * (h % 2)
            ch = 32 * (h // 2)
            hsl = slice(b64, b64 + 64)
            pslp = psum.tile([128, C], f32, tag="small", bufs=4)
            for c in range(C):
                csl = slice(c * W, (c + 1) * W)
                nc.tensor.matmul(pslp[:, c:c + 1], kT[t][hsl, csl],
                                 muT16[hsl, ch + c:ch + c + 1], start=True, stop=False)
                nc.tensor.matmul(pslp[:, c:c + 1], kT2[t][hsl, csl],
                                 ngh_sb[hsl, :], start=False, stop=True)
            explp = wkp.tile([128, C], f16, tag="explp", bufs=2)
            nc.scalar.activation(explp[:], pslp[:], AF.Exp)
            psu = psum.tile([D + 1, C], f32, tag="small", bufs=4)
            for c in range(C):
                nc.tensor.matmul(psu[:, c:c + 1], v_aug[:, c, h, :],
                                 explp[:, c:c + 1], start=True, stop=True)
            u16 = wkp.tile([D + 1, C], f16, tag="u16", bufs=2)
            nc.scalar.activation(u16[:], psu[:], AF.Copy, scale=1.0 / 16)
            psut = psum.tile([C, D + 1], f16, tag="smallb", bufs=2)
            nc.tensor.transpose(psut[:], u16[:], id16_sb[0:D + 1, 0:D + 1])
            nc.scalar.copy(rfa_aug[:, h, :], psut[:])
            rs32 = wkp.tile([32, 1], f32, tag="rs32", bufs=2)
            nc.vector.reciprocal(rs32[:], rfa_aug[:, h, D:D + 1])
            nc.vector.tensor_scalar_mul(rfa_aug[:, h, :], rfa_aug[:, h, :], rs32[:])

        # ---------------- phase 3: windowed attention ----------------
        # aon[t][:, w, :] holds attention output rows for window w in
        # natural (i, hd) layout; reuses the kT2 slots (tag match).
        aon = [bigp.tile([128, 8, 512], f16, tag=f"kT2{t}", name=f"aon{t}")
               for t in range(4)]
        for h in range(HPC):
            t, b64 = h // 2, 64 * (h % 2)
            ch = 32 * (h // 2)
            hsl = slice(b64, b64 + 64)
            for q4 in range(8):                       # quad of 4 windows
                w0 = q4 * 4
                psr4 = psum.tile([32, 512], f32, tag="small", bufs=4)
                nc.tensor.matmul(psr4[:], rfkbT16[hsl, ch:ch + 32],
                                 qT[t][hsl, w0 * W:(w0 + 4) * W], start=True, stop=True)
                expr4 = wkp.tile([32, 512], f16, tag="expr4", bufs=2)
                nc.scalar.activation(expr4[:], psr4[:], AF.Exp)
                for g0 in (w0, w0 + 2):
                    g1 = g0 + 1
                    q2 = qT[t][hsl, g0 * W:(g0 + 2) * W]
                    pss = psum.tile([128, 512], f32, tag="big", bufs=2)
                    if g0 > 0:
                        nc.tensor.matmul(pss[:, 0:128],
                                         kT[t][hsl, (g0 - 1) * W:g0 * W],
                                         qT[t][hsl, g0 * W:(g0 + 1) * W],
                                         start=True, stop=True)
                    nc.tensor.matmul(pss[:, 128:384], kT[t][hsl, g0 * W:(g0 + 1) * W],
                                     q2, start=True, stop=True)
                    nc.tensor.matmul(pss[:, 384:512], kT[t][hsl, g1 * W:(g1 + 1) * W],
                                     qT[t][hsl, g1 * W:(g1 + 1) * W],
                                     start=True, stop=True)
                    expd = wkp.tile([128, 512], f16, tag="expd", bufs=3)
                    if g0 > 0:
                        nc.scalar.activation(expd[:], pss[:], AF.Exp)
                    else:
                        nc.scalar.activation(expd[:, 128:512], pss[:, 128:512], AF.Exp)
                    nc.vector.tensor_tensor(out=expd[:, 128:256], in0=expd[:, 128:256],
                                            in1=mask_sb[:], op=ALU.mult)
                    nc.vector.tensor_tensor(out=expd[:, 384:512], in0=expd[:, 384:512],
                                            in1=mask_sb[:], op=ALU.mult)
                    for w in (g0, g1):
                        dbase = 128 + 256 * (w - g0)   # diag block columns in expd
                        wq = (w - w0) * 128            # this window's cols in expr4
                        pso = psum.tile([128, D + 1], f32, tag="small", bufs=4)
                        if w > 0:
                            nc.tensor.matmul(pso[:], expd[:, dbase - 128:dbase],
                                             v_aug[:, w - 1, h, :], start=True, stop=False)
                        nc.tensor.matmul(pso[:], expd[:, dbase:dbase + 128],
                                         v_aug[:, w, h, :], start=(w == 0),
                                         stop=(w == 0))
                        if w > 0:
                            nc.tensor.matmul(pso[:], expr4[0:w, wq:wq + 128],
                                             rfa_aug[0:w, h, :], start=False, stop=True)
                        rr = wkp.tile([128, 1], f32, tag="rr", bufs=3)
                        nc.vector.reciprocal(rr[:], pso[:, D:D + 1])
                        nc.vector.tensor_scalar(
                            out=aon[w // 8][:, w % 8, h * D:(h + 1) * D],
                            in0=pso[:, 0:D], scalar1=rr[:], scalar2=None, op0=ALU.mult)

        # ---------------- phase 4: output projection (partial) ----------------
        for ns in range(8):
            nsl = slice(ns * 512, (ns + 1) * 512)
            aotT = wkp.tile([128, 4, 512], f16, tag="aotT", bufs=2)
            for te in range(4):       # hd tile
                for wi in range(4):   # window within n-slice
                    w = ns * 4 + wi
                    pstr = psum.tile([128, 128], f16, tag="smallb", bufs=2)
                    nc.tensor.transpose(pstr[:], aon[w // 8][:, w % 8,
                                                            te * 128:(te + 1) * 128],
                                        id16_sb[:])
                    nc.vector.tensor_copy(aotT[:, te, wi * 128:(wi + 1) * 128], pstr[:])
            for e in range(8):
                ps = psum.tile([128, 512], f32, tag="big", bufs=2)
                for k in range(4):
                    nc.tensor.matmul(ps[:], wot_sb[:, k, e * 128:(e + 1) * 128],
                                     aotT[:, k, :], start=(k == 0), stop=(k == 3))
                stg = wkp.tile([128, 512], f16, tag="stg", bufs=3)
                nc.vector.tensor_copy(stg[:], ps[:])
                nc.sync.dma_start(outT[e * 128:(e + 1) * 128, nsl], stg[:])

    nc.compile()
    return nc


def _host_prep(inputs):
    q32 = np.asarray(inputs["query"], dtype=np.float32)
    Wq, bq = np.asarray(inputs["Wq"], np.float32), np.asarray(inputs["bq"], np.float32)
    Wk, bk = np.asarray(inputs["Wk"], np.float32), np.asarray(inputs["bk"], np.float32)
    Wv, bv = np.asarray(inputs["Wv"], np.float32), np.asarray(inputs["bv"], np.float32)
    Wo = np.asarray(inputs["Wo"], np.float32)
    f16 = np.float16

    j = np.arange(128)
    mask01 = (j[:, None] <= j[None, :]).astype(f16)          # [j_rel, i]
    ident = np.eye(128)

    common = {
        "mask01": mask01,
        "ident16": ident.astype(f16),
        "ident32": ident.astype(np.float32),
        "neghalf": np.full((128, 1), -SCALE / 2, f16),
        "ones16": np.ones((1, 128), f16),
        "epscol": np.full((128, 1), 1e-5, np.float32),
        "onesv": np.ones((128, 32 * 8), f16),
        "mubq_bc": np.broadcast_to(np.asarray(inputs["mu_q_b"], np.float32), (128, D)).copy(),
        "mubk_bc": np.broadcast_to(np.asarray(inputs["mu_k_b"], np.float32), (128, D)).copy(),
        "muqw": (np.asarray(inputs["mu_q_w"], np.float32).T / 128.0).astype(f16),
        "mukw": (np.asarray(inputs["mu_k_w"], np.float32).T / 128.0).astype(f16),
        "lnconst": np.concatenate([
            np.broadcast_to(np.asarray(inputs["mu_q_g"], np.float32), (128, D)),
            np.broadcast_to(np.asarray(inputs["mu_q_be"], np.float32), (128, D)),
            np.broadcast_to(np.asarray(inputs["mu_k_g"], np.float32), (128, D)),
            np.broadcast_to(np.asarray(inputs["mu_k_be"], np.float32), (128, D)),
        ], axis=1).copy(),
    }

    per_hg = []
    for hg in range(2):
        hs = slice(hg * 512, (hg + 1) * 512)
        wtc = np.concatenate([Wq[hs].T, Wk[hs].T, Wv[hs].T], axis=1)
        bqkc = np.concatenate([bq[hs] * SCALE, bk[hs]]).reshape(8, 128).T
        per_hg.append({
            "wt": np.ascontiguousarray(wtc).astype(f16),
            "bqk": np.ascontiguousarray(bqkc).astype(np.float32),
            "bvrow": bv[hs].reshape(1, 512).astype(f16),
            "wot": np.ascontiguousarray(Wo[:, hs].T).astype(f16),
        })

    in_maps = []
    for core in range(8):
        b, hg = core // 2, core % 2
        m = dict(common)
        m.update(per_hg[hg])
        m["xt"] = np.ascontiguousarray(q32[:, b, :].T).astype(f16)
        in_maps.append(m)
    return in_maps


def kernel(**inputs):
    if "nc" not in _CACHED:
        _CACHED["nc"] = _build_nc()
    nc = _CACHED["nc"]
    in_maps = _host_prep(inputs)
    run_kwargs = _CACHED.get("run_kwargs", {})
    res = run_bass_kernel_spmd(nc, in_maps, core_ids=list(range(8)), **run_kwargs)
    _CACHED["last_result"] = res

    bo = np.asarray(inputs["bo"], np.float32)
    out = np.empty((N, B, E), np.float32)
    for b in range(B):
        acc = res.results[2 * b]["outT"].astype(np.float32) \
            + res.results[2 * b + 1]["outT"].astype(np.float32)
        out[:, b, :] = acc.T + bo
    return out
